# revision 1
# baseline (speedup 1.0000x reference)
"""BestKRouter (top-2 MoE router) Trainium2 kernel.

logits = x @ W.T + b   [B,S,E]; top-2 over E; scatter into -inf; softmax.
Returns (router_out f32 [B,S,E], idxs int32 [B,S,2]).

Strategy: token-parallel over 8 cores (2048 tokens/core). Host prep per
core: transpose the token shard to [D, T] and split into fp16 hi/lo
(lo pre-scaled by 2048 so it stays in fp16 normal range). On device:
3-pass fp16 matmul (W^T chunks stationary, tokens moving) accumulating
logits^T [64, T] in PSUM (hi-pass and scaled lo-passes in separate
banks, combined + bias on DVE at fp32), PE-transpose of logits^T to
[T, 64], then top-2 via DVE max/max_index, softmax via ScalarE exp +
DVE reciprocal, and an iota==idx mask scatter.
"""

from contextlib import ExitStack

import numpy as np

import concourse.bass as bass
import concourse.tile as tile
from concourse import bacc, mybir
from concourse.bass_utils import run_bass_kernel_spmd
from concourse.masks import make_identity

N_CORES = 8
B, S, D, E = 4, 4096, 2048, 64
T_TOTAL = B * S
T_CORE = T_TOTAL // N_CORES  # 2048
NK = D // 128                # 16 contraction chunks
GROUP_T = 512
N_GROUPS = T_CORE // GROUP_T  # 4
NSUB = GROUP_T // 128         # 4
LO_SCALE = 2048.0

f32 = mybir.dt.float32
f16 = mybir.dt.float16
i32 = mybir.dt.int32
u32 = mybir.dt.uint32
Alu = mybir.AluOpType
Act = mybir.ActivationFunctionType


def build_nc(reps: int = 1, variant: str = "f16"):
    """Build the per-core Bass program. variant: "f16" (3-pass fp16 split)
    or "f32" (plain fp32 matmul)."""
    nc = bacc.Bacc(
        "TRN2", target_bir_lowering=False, debug=False, num_devices=N_CORES
    )
    if variant == "f16":
        xh_d = nc.dram_tensor("xh", [D, T_CORE], f16, kind="ExternalInput")
        xl_d = nc.dram_tensor("xl", [D, T_CORE], f16, kind="ExternalInput")
        wh_d = nc.dram_tensor("wh", [D, E], f16, kind="ExternalInput")
        wl_d = nc.dram_tensor("wl", [D, E], f16, kind="ExternalInput")
    else:
        xh_d = nc.dram_tensor("xh", [D, T_CORE], f32, kind="ExternalInput")
        wh_d = nc.dram_tensor("wh", [D, E], f32, kind="ExternalInput")
        xl_d = wl_d = None
    b_d = nc.dram_tensor("b", [E, 1], f32, kind="ExternalInput")
    rout_d = nc.dram_tensor("router", [T_CORE, E], f32, kind="ExternalOutput")
    idx_d = nc.dram_tensor("idxs", [T_CORE, 2], i32, kind="ExternalOutput")

    xdt = f16 if variant == "f16" else f32

    with tile.TileContext(nc) as tc, ExitStack() as ctx:
        cpool = ctx.enter_context(tc.tile_pool(name="const", bufs=1))
        xpool = ctx.enter_context(tc.tile_pool(name="x", bufs=2))
        spool = ctx.enter_context(tc.tile_pool(name="s", bufs=2))
        tiny = ctx.enter_context(tc.tile_pool(name="tiny", bufs=2))
        pps = ctx.enter_context(tc.tile_pool(name="ps", bufs=2, space="PSUM"))
        ppt = ctx.enter_context(tc.tile_pool(name="pt", bufs=2, space="PSUM"))

        ident = cpool.tile([E, E], f32)
        make_identity(nc, ident[:])
        iota_i = cpool.tile([128, E], i32)
        nc.gpsimd.iota(iota_i[:], pattern=[[1, E]], base=0, channel_multiplier=0)
        iota_f = cpool.tile([128, E], f32)
        nc.vector.tensor_copy(iota_f[:], iota_i[:])
        b_sb = cpool.tile([E, 1], f32)
        nc.sync.dma_start(b_sb[:], b_d[:])
        wh = cpool.tile([128, NK, E], xdt)
        nc.sync.dma_start(wh[:], wh_d.ap().rearrange("(k p) e -> p k e", p=128))
        if variant == "f16":
            wl = cpool.tile([128, NK, E], xdt)
            nc.sync.dma_start(wl[:], wl_d.ap().rearrange("(k p) e -> p k e", p=128))
        idx_stage = cpool.tile([128, T_CORE // 128, 2], i32)

        xh_ap = xh_d.ap().rearrange("(k p) t -> p k t", p=128)
        if variant == "f16":
            xl_ap = xl_d.ap().rearrange("(k p) t -> p k t", p=128)
        rout_ap = rout_d.ap().rearrange("(g j p) e -> g p j e", p=128, j=NSUB)
        idx_ap = idx_d.ap().rearrange("(n p) two -> p n two", p=128)

        def load(g):
            ts = g * GROUP_T
            xh_t = xpool.tile([128, NK, GROUP_T], xdt, tag="xh")
            nc.sync.dma_start(xh_t[:], xh_ap[:, :, ts:ts + GROUP_T])
            if variant == "f16":
                xl_t = xpool.tile([128, NK, GROUP_T], xdt, tag="xl")
                nc.sync.dma_start(xl_t[:], xl_ap[:, :, ts:ts + GROUP_T])
            else:
                xl_t = None
            return xh_t, xl_t

        def mm_hi(g, xh_t):
            ps_hi = pps.tile([E, GROUP_T], f32, tag="pshi")
            for k in range(NK):
                nc.tensor.matmul(
                    ps_hi[:], lhsT=wh[:, k, :], rhs=xh_t[:, k, :],
                    start=(k == 0), stop=(k == NK - 1),
                )
            return ps_hi

        def mm_lo(g, xh_t, xl_t):
            ps_lo = pps.tile([E, GROUP_T], f32, tag="pslo")
            for k in range(NK):
                nc.tensor.matmul(
                    ps_lo[:], lhsT=wl[:, k, :], rhs=xh_t[:, k, :],
                    start=(k == 0), stop=False,
                )
                nc.tensor.matmul(
                    ps_lo[:], lhsT=wh[:, k, :], rhs=xl_t[:, k, :],
                    start=False, stop=(k == NK - 1),
                )
            return ps_lo

        def combine(g, ps_hi, ps_lo):
            logT = spool.tile([E, GROUP_T], f32, tag="logT")
            if variant == "f16":
                u_sb = spool.tile([E, GROUP_T], f32, tag="u")
                nc.vector.tensor_scalar(
                    u_sb[:], ps_lo[:], 1.0 / LO_SCALE, b_sb[:],
                    op0=Alu.mult, op1=Alu.add,
                )
                nc.vector.tensor_add(logT[:], u_sb[:], ps_hi[:])
            else:
                nc.scalar.activation(
                    logT[:], ps_hi[:], Act.Identity, bias=b_sb[:], scale=1.0
                )
            return logT

        def post(g, logT):
            ps_t = ppt.tile([128, NSUB, E], f32, tag="pt")
            for j in range(NSUB):
                nc.tensor.transpose(
                    ps_t[:, j, :], logT[:, j * 128:(j + 1) * 128], ident[:]
                )
            logits = spool.tile([128, NSUB, E], f32, tag="logits")
            nc.scalar.copy(logits[:], ps_t[:])
            rstage = spool.tile([128, NSUB, E], f32, tag="rstage")
            for j in range(NSUB):
                L = logits[:, j, :]
                max8 = tiny.tile([128, 8], f32, tag="max8")
                nc.vector.max(max8[:], L)
                idx8 = tiny.tile([128, 8], u32, tag="idx8")
                nc.vector.max_index(idx8[:], max8[:], L)
                negm1 = tiny.tile([128, 1], f32, tag="negm1")
                nc.vector.tensor_scalar_mul(negm1[:], max8[:, 0:1], -1.0)
                e2 = tiny.tile([128, 1], f32, tag="e2")
                nc.scalar.activation(
                    e2[:], max8[:, 1:2], Act.Exp, bias=negm1[:], scale=1.0
                )
                Z = tiny.tile([128, 1], f32, tag="Z")
                nc.vector.tensor_scalar_add(Z[:], e2[:], 1.0)
                rZ = tiny.tile([128, 1], f32, tag="rZ")
                nc.vector.reciprocal(rZ[:], Z[:])
                idxf = tiny.tile([128, 2], f32, tag="idxf")
                nc.vector.tensor_copy(idxf[:], idx8[:, 0:2])
                t2 = tiny.tile([128, E], f32, tag="t2")
                nc.vector.tensor_scalar(
                    t2[:], iota_f[:], idxf[:, 1:2], e2[:],
                    op0=Alu.is_equal, op1=Alu.mult,
                )
                u2 = tiny.tile([128, E], f32, tag="u2")
                nc.vector.scalar_tensor_tensor(
                    u2[:], iota_f[:], idxf[:, 0:1], t2[:],
                    op0=Alu.is_equal, op1=Alu.add,
                )
                nc.scalar.mul(rstage[:, j, :], u2[:], rZ[:])
                nc.vector.tensor_copy(
                    idx_stage[:, g * NSUB + j, :], idx8[:, 0:2]
                )
            nc.sync.dma_start(rout_ap[g], rstage[:])

        def body(_iv):
            pending = None  # (g, logT) awaiting post-processing
            for g in range(N_GROUPS):
                xh_t, xl_t = load(g)
                ps_hi = mm_hi(g, xh_t)
                if pending is not None:
                    post(*pending)
                if variant == "f16":
                    ps_lo = mm_lo(g, xh_t, xl_t)
                else:
                    ps_lo = None
                logT = combine(g, ps_hi, ps_lo)
                pending = (g, logT)
            post(*pending)
            nc.sync.dma_start(idx_ap[:], idx_stage[:])

        if reps == 1:
            body(0)
        else:
            with tc.For_i(0, reps, 1) as iv:
                body(iv)

    nc.compile()
    return nc


def prep_inputs(x: np.ndarray, W: np.ndarray, b: np.ndarray, variant: str = "f16"):
    """Shard + host-side layout prep. Returns in_maps for the 8 cores."""
    xr = np.ascontiguousarray(x, dtype=np.float32).reshape(T_TOTAL, D)
    Wf = np.asarray(W, dtype=np.float32)
    bf = np.asarray(b, dtype=np.float32).reshape(E, 1)

    wT = np.ascontiguousarray(Wf.T)  # [D, E]
    if variant == "f16":
        wh = wT.astype(np.float16)
        wl = ((wT - wh.astype(np.float32)) * LO_SCALE).astype(np.float16)
    in_maps = []
    for c in range(N_CORES):
        shard = xr[c * T_CORE:(c + 1) * T_CORE]  # [T, D]
        xT = np.ascontiguousarray(shard.T)       # [D, T]
        if variant == "f16":
            xh = xT.astype(np.float16)
            xl = ((xT - xh.astype(np.float32)) * LO_SCALE).astype(np.float16)
            in_maps.append({"xh": xh, "xl": xl, "wh": wh, "wl": wl, "b": bf})
        else:
            in_maps.append({"xh": xT, "wh": wT, "b": bf})
    return in_maps


_NC_CACHE = {}


def kernel(x: np.ndarray, W: np.ndarray, b: np.ndarray):
    variant = "f16"
    key = (1, variant)
    if key not in _NC_CACHE:
        _NC_CACHE[key] = build_nc(reps=1, variant=variant)
    nc = _NC_CACHE[key]
    in_maps = prep_inputs(x, W, b, variant)
    res = run_bass_kernel_spmd(nc, in_maps, list(range(N_CORES))).results
    router = np.concatenate([r["router"] for r in res], axis=0).reshape(B, S, E)
    idxs = np.concatenate([r["idxs"] for r in res], axis=0).reshape(B, S, 2)
    return router.astype(np.float32), idxs.astype(np.int32)
